# revision 9
# baseline (speedup 1.0000x reference)
"""Trainium2 Bass kernel for nn_Capsule: capsule routing head.

Math: the einsum 'nco,pbo->bno' factorizes as xp[b,n,o] = W[n,o] * X[b,o]
with W = caps_weights.sum(c) (64x128) and X = x.sum(p) (256x128), so the
kernel is a memory-bound reduction of x (151 MB) followed by a tiny
per-batch routing loop (matmuls of size <= 128x64x128).

Sharding: data-parallel over batch (dim 1 of x), 32 batch elements per
core; caps_weights replicated; no cross-core communication.

Per-core pipeline (v2, tail-optimized):
  - 9 p-slabs of x (128, 4096) stream via both HWDGE rings which carry
    ONLY x (cst one-hot matrix built on-chip via memsets; caps_weights
    loaded on the gpsimd SWDGE ring) so both rings start immediately.
  - Reduction via fp32r matmuls with one-hot-column stationaries into 4
    psum banks, one per slab-group {0,1} {2,3} {4,5} {6,7,8}.  Mid
    groups are DMA'd in batch-halves and the last group in
    batch-eighths, interleaved so (group, b-range) work units complete
    progressively and only ~1.3us of matmul work remains after the
    final byte.  Group partials combine on DVE mid-stream; the tail
    does one strided reduce + one add.
  - Routing with fused DVE ops: scalar_tensor_tensor computes
    u=(S*rsum)*X in one op and u*u with accum_out gives the squared
    norm in one op; softmax denominator comes for free as a 129th
    ones-column in the S matmul; sqrt(q)=Exp(0.5*Ln(q)) on ACT (single
    pinned table); routing matmuls run fp32r via bitcast.
"""

import numpy as np

# ---- problem constants (hardcoded per contract) ----
P_TOT = 1152
BATCH = 256
O = 128
N_CAPS = 64
CAPS_DIM = 16
ITERATIONS = 3
N_CORES = 8
B_LOC = BATCH // N_CORES          # 32 batch elements per core
PT = P_TOT // 128                 # 9 p-slabs

_cache = {}


def _pin_act_table():
    """Force every ACT function onto the one table containing
    Exp+Ln+Square+Copy, so the kernel needs a single ACT_TABLE_LOAD."""
    import functools
    import concourse.hw_specs as hw_specs
    import concourse.bacc as bacc_mod

    if getattr(hw_specs.get_activation_tables, "_capsule_pinned", False):
        return
    orig = hw_specs.get_activation_tables

    @functools.cache
    def pinned(module_arch):
        tabs = orig(module_arch)
        keep = None
        for name, fns in tabs.items():
            names = {f.name for f in fns}
            if {"Exp", "Ln", "Square", "Copy", "Identity"} <= names:
                keep = name
                break
        if keep is None:
            return tabs
        return {n: (fns if n == keep else type(fns)()) for n, fns in tabs.items()}

    pinned._capsule_pinned = True
    hw_specs.get_activation_tables = pinned
    bacc_mod.get_activation_tables = pinned


def _build():
    _pin_act_table()
    import concourse.bacc as bacc
    import concourse.tile as tile
    import concourse.mybir as mybir
    from concourse.masks import make_identity

    f32 = mybir.dt.float32
    f32r = mybir.dt.float32r
    AX = mybir.AxisListType
    AF = mybir.ActivationFunctionType
    OP = mybir.AluOpType

    nc = bacc.Bacc(None, target_bir_lowering=False)

    # x declared f32r: same bytes as fp32, lets plain HWDGE DMAs feed the
    # fast fp32r matmul path with no cast.
    x_in = nc.dram_tensor("x", [P_TOT, B_LOC, O], f32r, kind="ExternalInput")
    w_in = nc.dram_tensor("caps_weights", [N_CAPS, CAPS_DIM, O], f32,
                          kind="ExternalInput")
    # one-hot stationary source: (128, 63) with ones in column 31, so
    # cst[:, 31-b : 63-b] is the one-hot-column-b matrix E_b.
    cst_in = nc.dram_tensor("cst", [128, 2 * B_LOC - 1], f32r,
                            kind="ExternalInput")
    out_d = nc.dram_tensor("out", [B_LOC, O], f32, kind="ExternalOutput")

    xv = x_in.rearrange("(t p) b o -> t p b o", p=128)   # (9, 128, 32, 128)

    # slab groups: sizes 2,2,2,3; one psum bank per group
    GROUPS = [(0, 2), (2, 2), (4, 2), (6, 3)]
    H = B_LOC // 2                 # batch half
    Q8 = B_LOC // 8                # batch eighth (4)

    with tile.TileContext(nc) as tc:
        with (
            tc.tile_pool(name="xin", bufs=1) as xpool,
            tc.tile_pool(name="wrk", bufs=1) as wrk,
            tc.tile_pool(name="small", bufs=1) as small,
            tc.tile_pool(name="ps", bufs=1, space="PSUM") as ps,
        ):
            engs = [nc.sync, nc.scalar]

            # group tiles in (s b o) layout: slab DMAs contiguous per
            # partition, matmul moving = strided (s,o) view per batch.
            xgs = []
            for g, (t0, gs) in enumerate(GROUPS):
                xg = xpool.tile([128, gs * B_LOC * O], f32r, tag=f"xg{g}",
                                name=f"xg{g}")
                xgs.append(xg)
            xg_vs = [xgs[g][:].rearrange("p (s b o) -> p s b o",
                                         b=B_LOC, o=O)
                     for g in range(len(GROUPS))]

            # ---- x DMAs first; both HWDGE rings carry only x ----
            # whole slabs 0,1 lead each ring; slabs 2-5 in batch-halves
            # (h0 on sync, h1 on scalar) so each half-unit's two slabs
            # finish together; group-3 slabs 6-8 in batch-eighths
            # interleaved by unit so unit q completes before unit q+1.
            nc.sync.dma_start(xg_vs[0][:, 0, :, :], xv[0])
            nc.scalar.dma_start(xg_vs[0][:, 1, :, :], xv[1])
            for t in (2, 3, 4, 5):
                g, sidx = (t - 2) // 2 + 1, (t - 2) % 2
                nc.sync.dma_start(xg_vs[g][:, sidx, :H, :], xv[t][:, :H, :])
                nc.scalar.dma_start(xg_vs[g][:, sidx, H:, :], xv[t][:, H:, :])
            k = 0
            for q in range(8):
                b0, b1 = q * Q8, (q + 1) * Q8
                for sidx, t in enumerate((6, 7, 8)):
                    eng = engs[0 if (k % 2 == 0 or k == 23) else 1]
                    eng.dma_start(xg_vs[3][:, sidx, b0:b1, :],
                                  xv[t][:, b0:b1, :])
                    k += 1

            # ---- capsule weights on the gpsimd SWDGE ring ----
            w_sb = wrk.tile([N_CAPS, CAPS_DIM * O], f32)
            nc.gpsimd.dma_start(w_sb[:], w_in.rearrange("n c o -> n (c o)"))

            # one-hot stationary source DMA'd on the gpsimd SWDGE ring so
            # the HWDGE rings carry only x (memset can't produce f32r and
            # the fp32r matmul verifier requires f32r-rounded producers).
            zpat = small.tile([128, 2 * B_LOC - 1], f32r)
            nc.gpsimd.dma_start(zpat[:], cst_in[:])

            ident = small.tile([128, 128], f32)
            make_identity(nc, ident[:])

            # ---- weight prep (overlaps the x stream) ----
            # w_no = caps_weights.sum(c) lands in wno1[:, :128]; column 128
            # is ones so the S-matmul also emits the softmax denominator.
            t1 = wrk.tile([N_CAPS, 8 * O], f32)
            nc.vector.tensor_tensor(t1[:], w_sb[:, :8 * O], w_sb[:, 8 * O:], OP.add)
            t2 = wrk.tile([N_CAPS, 4 * O], f32)
            nc.vector.tensor_tensor(t2[:], t1[:, :4 * O], t1[:, 4 * O:], OP.add)
            t3 = wrk.tile([N_CAPS, 2 * O], f32)
            nc.vector.tensor_tensor(t3[:], t2[:, :2 * O], t2[:, 2 * O:], OP.add)
            wno1 = wrk.tile([N_CAPS, O + 1], f32)
            nc.vector.tensor_tensor(wno1[:, :O], t3[:, :O], t3[:, O:], OP.add)
            nc.vector.memset(wno1[:, O:O + 1], 1.0)
            w_no = wno1[:, :O]

            ps_wt = ps.tile([O, N_CAPS], f32, tag="ps_t", name="ps_wt")
            nc.tensor.transpose(ps_wt[:], w_no, ident[:N_CAPS, :N_CAPS])
            wt_on = wrk.tile([O, N_CAPS], f32)          # W^T[o,n]
            nc.vector.tensor_copy(wt_on[:], ps_wt[:])
            # S0[b,o] = (1/64) sum_n W[n,o] for every b (uniform coeffs0)
            unif = small.tile([N_CAPS, B_LOC], f32)
            nc.vector.memset(unif[:], 1.0 / N_CAPS)
            ps_s0 = ps.tile([B_LOC, O], f32, tag="ps_s", name="ps_s0")
            nc.tensor.matmul(ps_s0[:], unif[:], w_no, start=True, stop=True)

            # ---- reduction: X[b,o] = sum_p x[p,b,o] ----
            # per (group, b): one matmul with one-hot-column stationary;
            # moving is the strided (s,o) view.  psum row b accumulates
            # the p-sum; other rows get += 0, so each bank needs one
            # accumulation group spanning its 32 matmuls.
            ps_gs = []
            for g, (t0, gs) in enumerate(GROUPS):
                ps_g = ps.tile([B_LOC, gs * O], f32, tag=f"ps_g{g}",
                               name=f"ps_g{g}")
                ps_gs.append(ps_g)
            mvs = [xgs[g][:].rearrange("p (s b o) -> p b s o", b=B_LOC, o=O)
                   for g in range(len(GROUPS))]

            def red_mm(g, b, start, stop):
                nc.tensor.matmul(
                    ps_gs[g][:], zpat[:, B_LOC - 1 - b: 2 * B_LOC - 1 - b],
                    mvs[g][:, b, :, :], start=start, stop=stop,
                    skip_group_check=True)

            for b in range(B_LOC):                      # group 0
                red_mm(0, b, b == 0, b == B_LOC - 1)
            for g in (1, 2):                            # groups 1,2: halves
                for b in range(B_LOC):
                    red_mm(g, b, b == 0, b == B_LOC - 1)
            for b in range(B_LOC):                      # group 3: eighths
                red_mm(3, b, b == 0, b == B_LOC - 1)

            # partial combines on DVE, all but the last mid-stream
            r0 = wrk.tile([B_LOC, O], f32)
            nc.vector.tensor_reduce(
                r0[:], ps_gs[0][:].rearrange("p (s o) -> p o s", o=O),
                AX.X, OP.add)
            r1 = wrk.tile([B_LOC, O], f32)
            nc.vector.tensor_reduce(
                r1[:], ps_gs[1][:].rearrange("p (s o) -> p o s", o=O),
                AX.X, OP.add)
            r01 = wrk.tile([B_LOC, O], f32)
            nc.vector.tensor_tensor(r01[:], r0[:], r1[:], OP.add)
            r2 = wrk.tile([B_LOC, O], f32)
            nc.vector.tensor_reduce(
                r2[:], ps_gs[2][:].rearrange("p (s o) -> p o s", o=O),
                AX.X, OP.add)
            r012 = wrk.tile([B_LOC, O], f32)
            nc.vector.tensor_tensor(r012[:], r01[:], r2[:], OP.add)
            r3 = wrk.tile([B_LOC, O], f32)
            nc.vector.tensor_reduce(
                r3[:], ps_gs[3][:].rearrange("p (s o) -> p o s", o=O),
                AX.X, OP.add)
            x32 = wrk.tile([B_LOC, O], f32)             # X[b,o]
            nc.vector.tensor_tensor(x32[:], r012[:], r3[:], OP.add)

            # ---- routing (b on partitions, fused DVE ops) ----
            u = wrk.tile([B_LOC, O], f32)
            sq = wrk.tile([B_LOC, O], f32)
            ux = wrk.tile([B_LOC, O], f32)
            tb = wrk.tile([B_LOC, O], f32)
            nsq = wrk.tile([B_LOC, 1], f32)
            lnq = wrk.tile([B_LOC, 1], f32)
            norm = wrk.tile([B_LOC, 1], f32)
            den = wrk.tile([B_LOC, 1], f32)
            rden = wrk.tile([B_LOC, 1], f32)
            rsum = wrk.tile([B_LOC, 1], f32)
            scale = wrk.tile([B_LOC, 1], f32)
            lg = wrk.tile([B_LOC, N_CAPS], f32)
            ex = wrk.tile([B_LOC, N_CAPS], f32)
            tT = wrk.tile([O, B_LOC], f32)
            exT = wrk.tile([N_CAPS, B_LOC], f32)

            for it in range(ITERATIONS):
                if it == 0:
                    # u0 = X * S0 (S0 read straight from psum)
                    nc.vector.tensor_tensor(u[:], x32[:], ps_s0[:], OP.mult)
                else:
                    # S|esum = exT^T @ [W | 1]; u = (S*rsum)*X in one op
                    ps_s = ps.tile([B_LOC, O + 1], f32, tag="ps_s",
                                   name=f"ps_s{it}")
                    nc.tensor.matmul(ps_s[:], exT[:], wno1[:],
                                     start=True, stop=True)
                    nc.vector.reciprocal(rsum[:], ps_s[:, O:O + 1])
                    nc.vector.scalar_tensor_tensor(
                        u[:], ps_s[:, :O], rsum[:], x32[:],
                        OP.mult, OP.mult)
                # nsq = sum_o u^2, fused square+accumulate
                nc.vector.scalar_tensor_tensor(
                    sq[:], u[:], 0.0, u[:], OP.bypass, OP.mult,
                    accum_out=nsq[:])
                nc.vector.tensor_scalar_add(den[:], nsq[:], 1.0)
                if it < ITERATIONS - 1:
                    nc.vector.tensor_tensor(ux[:], u[:], x32[:], OP.mult)
                # scale = sqrt(q)/(1+q); sqrt(q) = Exp(0.5*Ln(q)); the DVE
                # reciprocal of (1+q) overlaps the two ACT table lookups
                nc.scalar.activation(lnq[:], nsq[:], AF.Ln)
                nc.scalar.activation(norm[:], lnq[:], AF.Exp, scale=0.5)
                nc.vector.reciprocal(rden[:], den[:])
                nc.vector.tensor_tensor(scale[:], norm[:], rden[:], OP.mult)

                if it < ITERATIONS - 1:
                    # t = routed*X = scale*u*X ; delta[b,n] = sum_o t W^T
                    nc.vector.tensor_scalar_mul(tb[:], ux[:], scale[:])
                    ps_t = ps.tile([O, B_LOC], f32, tag="ps_t",
                                   name=f"ps_t{it}")
                    nc.tensor.transpose(ps_t[:], tb[:],
                                        ident[:B_LOC, :B_LOC])
                    nc.vector.tensor_copy(tT[:], ps_t[:])
                    ps_d = ps.tile([B_LOC, N_CAPS], f32, tag="ps_d",
                                   name=f"ps_d{it}")
                    nc.tensor.matmul(ps_d[:], tT[:], wt_on[:],
                                     start=True, stop=True)
                    # softmax over n (free axis, logits O(10): exp-safe);
                    # normalization deferred through rsum (matmul column)
                    if it == 0:
                        nc.scalar.activation(ex[:], ps_d[:], AF.Exp)
                        nc.vector.tensor_copy(lg[:], ps_d[:])
                    else:
                        lg2 = wrk.tile([B_LOC, N_CAPS], f32, tag="lg2")
                        nc.vector.tensor_tensor(lg2[:], ps_d[:], lg[:],
                                                OP.add)
                        nc.scalar.activation(ex[:], lg2[:], AF.Exp)
                    ps_ct = ps.tile([N_CAPS, B_LOC], f32, tag="ps_ct",
                                    name=f"ps_ct{it}")
                    nc.tensor.transpose(ps_ct[:], ex[:],
                                        ident[:B_LOC, :B_LOC])
                    nc.vector.tensor_copy(exT[:], ps_ct[:])
                else:
                    out_sb = wrk.tile([B_LOC, O], f32, tag="out_sb")
                    nc.vector.tensor_scalar_mul(out_sb[:], u[:], scale[:])
                    nc.sync.dma_start(out_d[:], out_sb[:])

    nc.compile()
    return nc


def run_with_results(x: np.ndarray, caps_weights: np.ndarray, **run_kwargs):
    """Run the SPMD kernel; returns (output (256,1,128), BassKernelResults)."""
    from concourse.bass_utils import run_bass_kernel_spmd

    if "nc" not in _cache:
        _cache["nc"] = _build()
    nc = _cache["nc"]

    x = np.ascontiguousarray(x, dtype=np.float32)
    caps_weights = np.ascontiguousarray(caps_weights, dtype=np.float32)
    cst = np.zeros((128, 2 * B_LOC - 1), dtype=np.float32)
    cst[:, B_LOC - 1] = 1.0

    in_maps = []
    for c in range(N_CORES):
        in_maps.append({
            "x": np.ascontiguousarray(x[:, c * B_LOC:(c + 1) * B_LOC, :]),
            "caps_weights": caps_weights,
            "cst": cst,
        })
    res = run_bass_kernel_spmd(nc, in_maps, core_ids=list(range(N_CORES)),
                               **run_kwargs)
    out = np.concatenate([res.results[c]["out"] for c in range(N_CORES)], axis=0)
    return out.reshape(BATCH, 1, O), res


def kernel(x: np.ndarray, caps_weights: np.ndarray) -> np.ndarray:
    out, _ = run_with_results(x, caps_weights)
    return out


# revision 13
# speedup vs baseline: 1.0021x; 1.0021x over previous
"""Trainium2 Bass kernel for nn_Capsule: capsule routing head.

Math: the einsum 'nco,pbo->bno' factorizes as xp[b,n,o] = W[n,o] * X[b,o]
with W = caps_weights.sum(c) (64x128) and X = x.sum(p) (256x128), so the
kernel is a memory-bound reduction of x (151 MB) followed by a tiny
per-batch routing loop (matmuls of size <= 128x64x128).

Sharding: data-parallel over batch (dim 1 of x), 32 batch elements per
core; caps_weights replicated; no cross-core communication.

Per-core pipeline (v2, tail-optimized):
  - 9 p-slabs of x (128, 4096) stream via both HWDGE rings which carry
    ONLY x (cst one-hot matrix built on-chip via memsets; caps_weights
    loaded on the gpsimd SWDGE ring) so both rings start immediately.
  - Reduction via fp32r matmuls with one-hot-column stationaries into 4
    psum banks, one per slab-group {0,1} {2,3} {4,5} {6,7,8}.  Mid
    groups are DMA'd in batch-halves and the last group in
    batch-eighths, interleaved so (group, b-range) work units complete
    progressively and only ~1.3us of matmul work remains after the
    final byte.  Group partials combine on DVE mid-stream; the tail
    does one strided reduce + one add.
  - Routing with fused DVE ops: scalar_tensor_tensor computes
    u=(S*rsum)*X in one op and u*u with accum_out gives the squared
    norm in one op; softmax denominator comes for free as a 129th
    ones-column in the S matmul; sqrt(q)=Exp(0.5*Ln(q)) on ACT (single
    pinned table); routing matmuls run fp32r via bitcast.
"""

import numpy as np

# ---- problem constants (hardcoded per contract) ----
P_TOT = 1152
BATCH = 256
O = 128
N_CAPS = 64
CAPS_DIM = 16
ITERATIONS = 3
N_CORES = 8
B_LOC = BATCH // N_CORES          # 32 batch elements per core
PT = P_TOT // 128                 # 9 p-slabs

_cache = {}


def _pin_act_table():
    """Force every ACT function onto the one table containing
    Exp+Ln+Square+Copy, so the kernel needs a single ACT_TABLE_LOAD."""
    import functools
    import concourse.hw_specs as hw_specs
    import concourse.bacc as bacc_mod

    if getattr(hw_specs.get_activation_tables, "_capsule_pinned", False):
        return
    orig = hw_specs.get_activation_tables

    @functools.cache
    def pinned(module_arch):
        tabs = orig(module_arch)
        keep = None
        for name, fns in tabs.items():
            names = {f.name for f in fns}
            if {"Exp", "Ln", "Square", "Copy", "Identity"} <= names:
                keep = name
                break
        if keep is None:
            return tabs
        return {n: (fns if n == keep else type(fns)()) for n, fns in tabs.items()}

    pinned._capsule_pinned = True
    hw_specs.get_activation_tables = pinned
    bacc_mod.get_activation_tables = pinned


def _build():
    _pin_act_table()
    import concourse.bacc as bacc
    import concourse.tile as tile
    import concourse.mybir as mybir
    from concourse.masks import make_identity

    f32 = mybir.dt.float32
    f32r = mybir.dt.float32r
    AX = mybir.AxisListType
    AF = mybir.ActivationFunctionType
    OP = mybir.AluOpType

    nc = bacc.Bacc(None, target_bir_lowering=False)

    # x declared f32r: same bytes as fp32, lets plain HWDGE DMAs feed the
    # fast fp32r matmul path with no cast.
    x_in = nc.dram_tensor("x", [P_TOT, B_LOC, O], f32r, kind="ExternalInput")
    w_in = nc.dram_tensor("caps_weights", [N_CAPS, CAPS_DIM, O], f32,
                          kind="ExternalInput")
    # one-hot stationary source: (128, 63) with ones in column 31, so
    # cst[:, 31-b : 63-b] is the one-hot-column-b matrix E_b.
    cst_in = nc.dram_tensor("cst", [128, 2 * B_LOC - 1], f32r,
                            kind="ExternalInput")
    out_d = nc.dram_tensor("out", [B_LOC, O], f32, kind="ExternalOutput")

    xv = x_in.rearrange("(t p) b o -> t p b o", p=128)   # (9, 128, 32, 128)

    # slab groups: sizes 2,2,2,3; one psum bank per group
    GROUPS = [(0, 2), (2, 2), (4, 2), (6, 3)]
    H = B_LOC // 2                 # batch half
    Q8 = B_LOC // 8                # batch eighth (4)

    with tile.TileContext(nc) as tc:
        with (
            tc.tile_pool(name="xin", bufs=1) as xpool,
            tc.tile_pool(name="wrk", bufs=1) as wrk,
            tc.tile_pool(name="small", bufs=1) as small,
            tc.tile_pool(name="ps", bufs=1, space="PSUM") as ps,
        ):
            engs = [nc.sync, nc.scalar]

            # group tiles in (s b o) layout: slab DMAs contiguous per
            # partition, matmul moving = strided (s,o) view per batch.
            xgs = []
            for g, (t0, gs) in enumerate(GROUPS):
                xg = xpool.tile([128, gs * B_LOC * O], f32r, tag=f"xg{g}",
                                name=f"xg{g}")
                xgs.append(xg)
            xg_vs = [xgs[g][:].rearrange("p (s b o) -> p s b o",
                                         b=B_LOC, o=O)
                     for g in range(len(GROUPS))]

            # ---- DMAs: both HWDGE rings stream x immediately; cst leads
            # scalar where it rides out the ACT table load that blocks
            # that ring's start anyway; w slots in mid-sync (its weight
            # prep isn't needed until routing).
            zpat = small.tile([128, 2 * B_LOC - 1], f32r)
            nc.scalar.dma_start(zpat[:], cst_in[:])

            # whole slabs 0,1 lead each ring; slabs 2-5 in batch-halves
            # (h0 on sync, h1 on scalar) so each half-unit's two slabs
            # finish together; group-3 slabs 6-8 in batch-eighths
            # interleaved by unit so unit q completes before unit q+1.
            nc.sync.dma_start(xg_vs[0][:, 0, :, :], xv[0])
            nc.scalar.dma_start(xg_vs[0][:, 1, :, :], xv[1])
            for t in (2, 3, 4, 5):
                g, sidx = (t - 2) // 2 + 1, (t - 2) % 2
                nc.sync.dma_start(xg_vs[g][:, sidx, :H, :], xv[t][:, :H, :])
                nc.scalar.dma_start(xg_vs[g][:, sidx, H:, :], xv[t][:, H:, :])
            w_sb = wrk.tile([N_CAPS, CAPS_DIM * O], f32)
            nc.sync.dma_start(w_sb[:], w_in.rearrange("n c o -> n (c o)"))
            k = 0
            for q in range(8):
                b0, b1 = q * Q8, (q + 1) * Q8
                for sidx, t in enumerate((6, 7, 8)):
                    # 11 pieces on sync, 13 on scalar: balances ring ends
                    # given w on sync and scalar's act-table-delayed start
                    eng = engs[0 if (k % 2 == 0 and k != 22) else 1]
                    eng.dma_start(xg_vs[3][:, sidx, b0:b1, :],
                                  xv[t][:, b0:b1, :])
                    k += 1

            ident = small.tile([128, 128], f32)
            make_identity(nc, ident[:])

            # ---- PE warm-up: the HAM throttles the PE array to half
            # duty after long idle; burn cheap dummy matmuls on the
            # identity tile (never read) so the first real matmuls run
            # at full rate.
            ps_dmy = ps.tile([B_LOC, O], f32, tag="ps_d", name="ps_dmy")

            def warm_mm():
                nc.tensor.matmul(ps_dmy[:], ident[:, :B_LOC], ident[:],
                                 start=True, stop=True,
                                 skip_group_check=True)

            for i in range(24):
                warm_mm()

            # ---- weight prep (overlaps the x stream) ----
            # w_no = caps_weights.sum(c) lands in wno1[:, :128]; column 128
            # is ones so the S-matmul also emits the softmax denominator.
            t1 = wrk.tile([N_CAPS, 8 * O], f32)
            nc.vector.tensor_tensor(t1[:], w_sb[:, :8 * O], w_sb[:, 8 * O:], OP.add)
            t2 = wrk.tile([N_CAPS, 4 * O], f32)
            nc.vector.tensor_tensor(t2[:], t1[:, :4 * O], t1[:, 4 * O:], OP.add)
            t3 = wrk.tile([N_CAPS, 2 * O], f32)
            nc.vector.tensor_tensor(t3[:], t2[:, :2 * O], t2[:, 2 * O:], OP.add)
            wno1 = wrk.tile([N_CAPS, O + 1], f32)
            nc.vector.tensor_tensor(wno1[:, :O], t3[:, :O], t3[:, O:], OP.add)
            nc.vector.memset(wno1[:, O:O + 1], 1.0)
            w_no = wno1[:, :O]

            ps_wt = ps.tile([O, N_CAPS], f32, tag="ps_t", name="ps_wt")
            nc.tensor.transpose(ps_wt[:], w_no, ident[:N_CAPS, :N_CAPS])
            wt_on = wrk.tile([O, N_CAPS], f32)          # W^T[o,n]
            nc.vector.tensor_copy(wt_on[:], ps_wt[:])
            # S0[b,o] = (1/64) sum_n W[n,o] for every b (uniform coeffs0)
            unif = small.tile([N_CAPS, B_LOC], f32)
            nc.vector.memset(unif[:], 1.0 / N_CAPS)
            ps_s0 = ps.tile([B_LOC, O], f32, tag="ps_s", name="ps_s0")
            nc.tensor.matmul(ps_s0[:], unif[:], w_no, start=True, stop=True)

            # ---- reduction: X[b,o] = sum_p x[p,b,o] ----
            # per (group, b): one matmul with one-hot-column stationary;
            # moving is the strided (s,o) view.  psum row b accumulates
            # the p-sum; other rows get += 0, so each bank needs one
            # accumulation group spanning its 32 matmuls.
            ps_gs = []
            for g, (t0, gs) in enumerate(GROUPS):
                ps_g = ps.tile([B_LOC, gs * O], f32, tag=f"ps_g{g}",
                               name=f"ps_g{g}")
                ps_gs.append(ps_g)
            mvs = [xgs[g][:].rearrange("p (s b o) -> p b s o", b=B_LOC, o=O)
                   for g in range(len(GROUPS))]

            def red_mm(g, b, start, stop):
                nc.tensor.matmul(
                    ps_gs[g][:], zpat[:, B_LOC - 1 - b: 2 * B_LOC - 1 - b],
                    mvs[g][:, b, :, :], start=start, stop=stop,
                    skip_group_check=True)

            for b in range(B_LOC):                      # group 0
                red_mm(0, b, b == 0, b == B_LOC - 1)
            for g in (1, 2):                            # groups 1,2: halves
                for b in range(B_LOC):
                    red_mm(g, b, b == 0, b == B_LOC - 1)
            for b in range(B_LOC):                      # group 3: eighths
                red_mm(3, b, b == 0, b == B_LOC - 1)
                # the tail units arrive ~1.6us apart but cost only ~1.3us
                # of PE; keep the HAM duty cycle up through the gaps so
                # the tail and routing matmuls run at full rate
                if b % Q8 == Q8 - 1 and b != B_LOC - 1:
                    warm_mm()
                    warm_mm()

            # partial combines on DVE, all but the last mid-stream
            r0 = wrk.tile([B_LOC, O], f32)
            nc.vector.tensor_reduce(
                r0[:], ps_gs[0][:].rearrange("p (s o) -> p o s", o=O),
                AX.X, OP.add)
            r1 = wrk.tile([B_LOC, O], f32)
            nc.vector.tensor_reduce(
                r1[:], ps_gs[1][:].rearrange("p (s o) -> p o s", o=O),
                AX.X, OP.add)
            r01 = wrk.tile([B_LOC, O], f32)
            nc.vector.tensor_tensor(r01[:], r0[:], r1[:], OP.add)
            r2 = wrk.tile([B_LOC, O], f32)
            nc.vector.tensor_reduce(
                r2[:], ps_gs[2][:].rearrange("p (s o) -> p o s", o=O),
                AX.X, OP.add)
            r012 = wrk.tile([B_LOC, O], f32)
            nc.vector.tensor_tensor(r012[:], r01[:], r2[:], OP.add)
            r3 = wrk.tile([B_LOC, O], f32)
            nc.vector.tensor_reduce(
                r3[:], ps_gs[3][:].rearrange("p (s o) -> p o s", o=O),
                AX.X, OP.add)
            x32 = wrk.tile([B_LOC, O], f32)             # X[b,o]
            nc.vector.tensor_tensor(x32[:], r012[:], r3[:], OP.add)

            # ---- routing (b on partitions, fused DVE ops) ----
            u = wrk.tile([B_LOC, O], f32)
            sq = wrk.tile([B_LOC, O], f32)
            ux = wrk.tile([B_LOC, O], f32)
            tb = wrk.tile([B_LOC, O], f32)
            nsq = wrk.tile([B_LOC, 1], f32)
            lnq = wrk.tile([B_LOC, 1], f32)
            norm = wrk.tile([B_LOC, 1], f32)
            den = wrk.tile([B_LOC, 1], f32)
            rden = wrk.tile([B_LOC, 1], f32)
            rsum = wrk.tile([B_LOC, 1], f32)
            scale = wrk.tile([B_LOC, 1], f32)
            lg = wrk.tile([B_LOC, N_CAPS], f32)
            ex = wrk.tile([B_LOC, N_CAPS], f32)
            tT = wrk.tile([O, B_LOC], f32)
            exT = wrk.tile([N_CAPS, B_LOC], f32)

            for it in range(ITERATIONS):
                if it == 0:
                    # u0 = X * S0 (S0 read straight from psum)
                    nc.vector.tensor_tensor(u[:], x32[:], ps_s0[:], OP.mult)
                else:
                    # S|esum = exT^T @ [W | 1]; u = (S*rsum)*X in one op
                    ps_s = ps.tile([B_LOC, O + 1], f32, tag="ps_s",
                                   name=f"ps_s{it}")
                    nc.tensor.matmul(ps_s[:], exT[:], wno1[:],
                                     start=True, stop=True)
                    nc.vector.reciprocal(rsum[:], ps_s[:, O:O + 1])
                    nc.vector.scalar_tensor_tensor(
                        u[:], ps_s[:, :O], rsum[:], x32[:],
                        OP.mult, OP.mult)
                # nsq = sum_o u^2, fused square+accumulate
                nc.vector.scalar_tensor_tensor(
                    sq[:], u[:], 0.0, u[:], OP.bypass, OP.mult,
                    accum_out=nsq[:])
                nc.vector.tensor_scalar_add(den[:], nsq[:], 1.0)
                if it < ITERATIONS - 1:
                    nc.vector.tensor_tensor(ux[:], u[:], x32[:], OP.mult)
                # scale = sqrt(q)/(1+q); sqrt(q) = Exp(0.5*Ln(q)); the DVE
                # reciprocal of (1+q) overlaps the two ACT table lookups
                nc.scalar.activation(lnq[:], nsq[:], AF.Ln)
                nc.scalar.activation(norm[:], lnq[:], AF.Exp, scale=0.5)
                nc.vector.reciprocal(rden[:], den[:])
                nc.vector.tensor_tensor(scale[:], norm[:], rden[:], OP.mult)

                if it < ITERATIONS - 1:
                    # t = routed*X = scale*u*X ; delta[b,n] = sum_o t W^T
                    nc.vector.tensor_scalar_mul(tb[:], ux[:], scale[:])
                    ps_t = ps.tile([O, B_LOC], f32, tag="ps_t",
                                   name=f"ps_t{it}")
                    nc.tensor.transpose(ps_t[:], tb[:],
                                        ident[:B_LOC, :B_LOC])
                    nc.vector.tensor_copy(tT[:], ps_t[:])
                    ps_d = ps.tile([B_LOC, N_CAPS], f32, tag="ps_d",
                                   name=f"ps_d{it}")
                    nc.tensor.matmul(ps_d[:], tT[:], wt_on[:],
                                     start=True, stop=True)
                    # softmax over n (free axis, logits O(10): exp-safe);
                    # normalization deferred through rsum (matmul column)
                    if it == 0:
                        nc.scalar.activation(ex[:], ps_d[:], AF.Exp)
                        nc.vector.tensor_copy(lg[:], ps_d[:])
                    else:
                        lg2 = wrk.tile([B_LOC, N_CAPS], f32, tag="lg2")
                        nc.vector.tensor_tensor(lg2[:], ps_d[:], lg[:],
                                                OP.add)
                        nc.scalar.activation(ex[:], lg2[:], AF.Exp)
                    ps_ct = ps.tile([N_CAPS, B_LOC], f32, tag="ps_ct",
                                    name=f"ps_ct{it}")
                    nc.tensor.transpose(ps_ct[:], ex[:],
                                        ident[:B_LOC, :B_LOC])
                    nc.vector.tensor_copy(exT[:], ps_ct[:])
                else:
                    out_sb = wrk.tile([B_LOC, O], f32, tag="out_sb")
                    nc.vector.tensor_scalar_mul(out_sb[:], u[:], scale[:])
                    nc.sync.dma_start(out_d[:], out_sb[:])

    nc.compile()
    return nc


def run_with_results(x: np.ndarray, caps_weights: np.ndarray, **run_kwargs):
    """Run the SPMD kernel; returns (output (256,1,128), BassKernelResults)."""
    from concourse.bass_utils import run_bass_kernel_spmd

    if "nc" not in _cache:
        _cache["nc"] = _build()
    nc = _cache["nc"]

    x = np.ascontiguousarray(x, dtype=np.float32)
    caps_weights = np.ascontiguousarray(caps_weights, dtype=np.float32)
    cst = np.zeros((128, 2 * B_LOC - 1), dtype=np.float32)
    cst[:, B_LOC - 1] = 1.0

    in_maps = []
    for c in range(N_CORES):
        in_maps.append({
            "x": np.ascontiguousarray(x[:, c * B_LOC:(c + 1) * B_LOC, :]),
            "caps_weights": caps_weights,
            "cst": cst,
        })
    res = run_bass_kernel_spmd(nc, in_maps, core_ids=list(range(N_CORES)),
                               **run_kwargs)
    out = np.concatenate([res.results[c]["out"] for c in range(N_CORES)], axis=0)
    return out.reshape(BATCH, 1, O), res


def kernel(x: np.ndarray, caps_weights: np.ndarray) -> np.ndarray:
    out, _ = run_with_results(x, caps_weights)
    return out
